# revision 5
# baseline (speedup 1.0000x reference)
"""BiRNN LM kernel for Trainium2, 8 NeuronCores.

Strategy (data-parallel over batch):
  - batch B=32 is split 4 columns per core; each core computes its
    [S=128, BL=4] slice end-to-end: embedding gather, both RNN scans,
    the vocab projection and log_softmax, writing [512, 50257] fp32.
  - the big [rows, V] matmul folds b_out in as a 33rd contraction row
    (ones row in the feature matrix), runs in bf16, and is done twice:
    pass A feeds exp()+accumulate to get the per-row logsumexp, pass B
    recomputes logits and subtracts L while staging ~2MB output DMAs.
  - no max-subtraction is needed: |logits| <= 33 * 1.0 * (1/sqrt(V)) ~ 0.15,
    so exp() is stable and log_softmax(x) = x - log(sum(exp(x))) exactly.
"""

from contextlib import ExitStack

import ml_dtypes
import numpy as np

import concourse.bass as bass
import concourse.tile as tile
from concourse import bacc
from concourse import mybir
from concourse.bass_utils import run_bass_kernel_spmd
from concourse.masks import make_identity

S, B, V = 128, 32, 50257
EMB, HID = 32, 16
NCORES = 8
BL = B // NCORES          # 4 batch columns per core
R = S * BL                # 512 rows per core (row r = t*BL + b)
KF = 2 * HID + 1          # 33 = contraction rows of the vocab matmul
CHUNK = 512               # vocab columns per matmul (one PSUM bank)
NCH = (V + CHUNK - 1) // CHUNK   # 99
STAGE = 8 * CHUNK         # vocab columns per output DMA (4096)
ROWT = R // 128           # 4 row-tiles of 128 rows

_F32 = mybir.dt.float32
_BF16 = mybir.dt.bfloat16
_I32 = mybir.dt.int32
_AF = mybir.ActivationFunctionType
_ALU = mybir.AluOpType

_CACHE: dict = {}


def _emit_rep(nc, tc, pools, aps, rep):
    (const, gather, scr, stats, ostage) = pools
    (embtab, idx, h0lrT, h0rlT, out, wb_sb, wxlr_sb, whlr_sb, blr_sb,
     wxrl_sb, whrl_sb, brl_sb, ident) = aps

    embT = const.tile([EMB, R], _F32, tag="embT")   # emb[t,b]^T at col t*BL+b
    hlr = const.tile([HID, R], _F32, tag="hlr")     # hLR[t]^T at col t*BL+b
    hrl = const.tile([HID, R], _F32, tag="hrl")     # hRL[S-1-t]^T at col t*BL+b
    fb = const.tile([KF, R], _BF16, tag="fb")       # [hLR; hRL_rev; ones]

    with tc.tile_pool(name=f"psum_pro{rep}", bufs=2, space="PSUM") as psum_pro:
        # ---- embedding gather: rows -> [128, EMB] tiles, PE-transpose into embT
        for g in range(R // 128):
            it = gather.tile([128, 1], _I32, tag="it")
            nc.sync.dma_start(it[:], idx[g * 128 : (g + 1) * 128, :])
            en = gather.tile([128, EMB], _F32, tag="en")
            nc.gpsimd.indirect_dma_start(
                out=en[:],
                out_offset=None,
                in_=embtab[:],
                in_offset=bass.IndirectOffsetOnAxis(ap=it[:, :1], axis=0),
            )
            pt = psum_pro.tile([EMB, 128], _F32, tag="pt")
            nc.tensor.transpose(out=pt[:], in_=en[:], identity=ident[:])
            nc.vector.tensor_copy(embT[:, g * 128 : (g + 1) * 128], pt[:])

        # ---- initial hidden states
        nc.sync.dma_start(hlr[:, 0:BL], h0lrT[:])
        nc.sync.dma_start(hrl[:, (S - 1) * BL : S * BL], h0rlT[:])

        # ---- the two scans, interleaved (independent chains)
        # LR step t:  hLR[t] = tanh(Wx@emb[t-1] + Wh@hLR[t-1] + b)
        # RL step k:  hRL[k] = tanh(Wx@emb[S-k] + Wh@hRL[k-1] + b);
        #             hRL[k] lives at col t=S-1-k, hRL[k-1] at col t=S-k.
        for s_ in range(1, S):
            plr = psum_pro.tile([HID, BL], _F32, tag="plr")
            nc.tensor.matmul(
                plr[:], wxlr_sb[:], embT[:, (s_ - 1) * BL : s_ * BL],
                start=True, stop=False,
            )
            nc.tensor.matmul(
                plr[:], whlr_sb[:], hlr[:, (s_ - 1) * BL : s_ * BL],
                start=False, stop=True,
            )
            nc.scalar.activation(
                hlr[:, s_ * BL : (s_ + 1) * BL], plr[:], _AF.Tanh,
                bias=blr_sb[:, 0:1],
            )
            tcol = S - 1 - s_
            prl = psum_pro.tile([HID, BL], _F32, tag="prl")
            nc.tensor.matmul(
                prl[:], wxrl_sb[:], embT[:, (S - s_) * BL : (S - s_ + 1) * BL],
                start=True, stop=False,
            )
            nc.tensor.matmul(
                prl[:], whrl_sb[:], hrl[:, (S - s_) * BL : (S - s_ + 1) * BL],
                start=False, stop=True,
            )
            nc.scalar.activation(
                hrl[:, tcol * BL : (tcol + 1) * BL], prl[:], _AF.Tanh,
                bias=brl_sb[:, 0:1],
            )

        # ---- assemble bf16 feature matrix [33, R] (cast + ones row)
        nc.gpsimd.dma_start(fb[0:HID, :], hlr[:, :])
        nc.gpsimd.dma_start(fb[HID : 2 * HID, :], hrl[:, :])
        nc.vector.memset(fb[2 * HID : KF, :], 1.0)

    with tc.tile_pool(name=f"psum_a{rep}", bufs=2, space="PSUM") as psum_a, \
         tc.tile_pool(name=f"psum_b{rep}", bufs=2, space="PSUM") as psum_b:
        # ---- vocab projection + log_softmax, 4 row-tiles of 128 rows
        for i in range(ROWT):
            lhs = fb[:, i * 128 : (i + 1) * 128]
            sums = stats.tile([128, NCH], _F32, tag="sums")
            # pass A: logits -> exp -> per-chunk sums
            for j in range(NCH):
                c0 = j * CHUNK
                n = min(CHUNK, V - c0)
                pa = psum_a.tile([128, CHUNK], _F32, tag="pa")
                nc.tensor.matmul(
                    pa[:, :n], lhs, wb_sb[:, c0 : c0 + n], start=True, stop=True
                )
                sc = scr.tile([128, CHUNK], _BF16, tag="sc")
                nc.scalar.activation(
                    sc[:, :n], pa[:, :n], _AF.Exp, accum_out=sums[:, j : j + 1]
                )
            tot = stats.tile([128, 1], _F32, tag="tot")
            nc.vector.tensor_reduce(
                tot[:], sums[:], axis=mybir.AxisListType.X, op=_ALU.add
            )
            lse = stats.tile([128, 1], _F32, tag="lse")
            nc.scalar.activation(lse[:], tot[:], _AF.Ln)
            # pass B: recompute logits, subtract logsumexp, stage + DMA out
            col = 0
            while col < V:
                w = min(STAGE, V - col)
                ob = ostage.tile([128, STAGE], _F32, tag="ob")
                off = 0
                while off < w:
                    n = min(CHUNK, w - off)
                    pb = psum_b.tile([128, CHUNK], _F32, tag="pb")
                    nc.tensor.matmul(
                        pb[:, :n], lhs, wb_sb[:, col + off : col + off + n],
                        start=True, stop=True,
                    )
                    nc.vector.tensor_scalar(
                        ob[:, off : off + n], pb[:, :n], lse[:], None, _ALU.subtract
                    )
                    off += n
                nc.sync.dma_start(
                    out[i * 128 : (i + 1) * 128, col : col + w], ob[:, :w]
                )
                col += w


def _build_nc(repeats: int = 1) -> bass.Bass:
    nc = bacc.Bacc("TRN2", target_bir_lowering=False, debug=False)

    embtab = nc.dram_tensor("embtab", [V, EMB], _F32, kind="ExternalInput").ap()
    idx = nc.dram_tensor("idx", [R, 1], _I32, kind="ExternalInput").ap()
    wxt_lr = nc.dram_tensor("wxt_lr", [EMB, HID], _F32, kind="ExternalInput").ap()
    wht_lr = nc.dram_tensor("wht_lr", [HID, HID], _F32, kind="ExternalInput").ap()
    b_lr = nc.dram_tensor("b_lr", [HID, 1], _F32, kind="ExternalInput").ap()
    wxt_rl = nc.dram_tensor("wxt_rl", [EMB, HID], _F32, kind="ExternalInput").ap()
    wht_rl = nc.dram_tensor("wht_rl", [HID, HID], _F32, kind="ExternalInput").ap()
    b_rl = nc.dram_tensor("b_rl", [HID, 1], _F32, kind="ExternalInput").ap()
    h0lrT = nc.dram_tensor("h0lrT", [HID, BL], _F32, kind="ExternalInput").ap()
    h0rlT = nc.dram_tensor("h0rlT", [HID, BL], _F32, kind="ExternalInput").ap()
    wb = nc.dram_tensor("wb", [KF, V], _BF16, kind="ExternalInput").ap()
    out = nc.dram_tensor("out", [R, V], _F32, kind="ExternalOutput").ap()

    with tile.TileContext(nc) as tc, ExitStack() as ctx:
        const = ctx.enter_context(tc.tile_pool(name="const", bufs=1))
        gather = ctx.enter_context(tc.tile_pool(name="gather", bufs=2))
        scr = ctx.enter_context(tc.tile_pool(name="scr", bufs=2))
        stats = ctx.enter_context(tc.tile_pool(name="stats", bufs=2))
        ostage = ctx.enter_context(tc.tile_pool(name="ostage", bufs=3))

        # ---- constants into SBUF (loaded once)
        wb_sb = const.tile([KF, V], _BF16)
        nc.sync.dma_start(wb_sb[:], wb[:])
        wxlr_sb = const.tile([EMB, HID], _F32)
        nc.sync.dma_start(wxlr_sb[:], wxt_lr[:])
        whlr_sb = const.tile([HID, HID], _F32)
        nc.sync.dma_start(whlr_sb[:], wht_lr[:])
        blr_sb = const.tile([HID, 1], _F32)
        nc.sync.dma_start(blr_sb[:], b_lr[:])
        wxrl_sb = const.tile([EMB, HID], _F32)
        nc.sync.dma_start(wxrl_sb[:], wxt_rl[:])
        whrl_sb = const.tile([HID, HID], _F32)
        nc.sync.dma_start(whrl_sb[:], wht_rl[:])
        brl_sb = const.tile([HID, 1], _F32)
        nc.sync.dma_start(brl_sb[:], b_rl[:])
        ident = const.tile([128, 128], _F32)
        make_identity(nc, ident[:])

        pools = (const, gather, scr, stats, ostage)
        aps = (embtab, idx, h0lrT, h0rlT, out, wb_sb, wxlr_sb, whlr_sb,
               blr_sb, wxrl_sb, whrl_sb, brl_sb, ident)
        for rep in range(repeats):
            _emit_rep(nc, tc, pools, aps, rep)

    nc.compile()
    return nc


def _get_nc(repeats: int = 1) -> bass.Bass:
    key = f"nc{repeats}"
    if key not in _CACHE:
        _CACHE[key] = _build_nc(repeats)
    return _CACHE[key]


def _make_in_maps(inputs: dict) -> list[dict]:
    ib = np.asarray(inputs["input_batch"]).astype(np.int32)          # [S, B]
    emb = np.ascontiguousarray(np.asarray(inputs["embedding"], dtype=np.float32))
    w_lr = np.asarray(inputs["W_lr"], dtype=np.float32)              # [HID, EMB+HID]
    w_rl = np.asarray(inputs["W_rl"], dtype=np.float32)
    b_lr = np.asarray(inputs["b_lr"], dtype=np.float32)
    b_rl = np.asarray(inputs["b_rl"], dtype=np.float32)
    w_out = np.asarray(inputs["W_out"], dtype=np.float32)            # [V, 2*HID]
    b_out = np.asarray(inputs["b_out"], dtype=np.float32)
    h0_lr = np.asarray(inputs["h0_lr"], dtype=np.float32)            # [B, HID]
    h0_rl = np.asarray(inputs["h0_rl"], dtype=np.float32)

    shared = {
        "embtab": emb,
        "wxt_lr": np.ascontiguousarray(w_lr[:, :EMB].T),
        "wht_lr": np.ascontiguousarray(w_lr[:, EMB:].T),
        "b_lr": np.ascontiguousarray(b_lr[:, None]),
        "wxt_rl": np.ascontiguousarray(w_rl[:, :EMB].T),
        "wht_rl": np.ascontiguousarray(w_rl[:, EMB:].T),
        "b_rl": np.ascontiguousarray(b_rl[:, None]),
        "wb": np.ascontiguousarray(
            np.concatenate([w_out.T, b_out[None, :]], axis=0)
        ).astype(ml_dtypes.bfloat16),
    }
    in_maps = []
    for c in range(NCORES):
        cols = slice(c * BL, (c + 1) * BL)
        in_maps.append(
            dict(
                shared,
                idx=np.ascontiguousarray(ib[:, cols].reshape(R, 1)),
                h0lrT=np.ascontiguousarray(h0_lr[cols, :].T),
                h0rlT=np.ascontiguousarray(h0_rl[cols, :].T),
            )
        )
    return in_maps


def _run(inputs: dict, repeats: int = 1, **spmd_kwargs):
    nc = _get_nc(repeats)
    res = run_bass_kernel_spmd(
        nc, _make_in_maps(inputs), core_ids=list(range(NCORES)), **spmd_kwargs
    )
    outs = [res.results[c]["out"].reshape(S, BL, V) for c in range(NCORES)]
    return np.concatenate(outs, axis=1), res


def kernel(**inputs) -> np.ndarray:
    full, _ = _run(inputs)
    return full


# revision 9
# speedup vs baseline: 1.0655x; 1.0655x over previous
"""BiRNN LM kernel for Trainium2, 8 NeuronCores.

Strategy (data-parallel over batch):
  - batch B=32 is split 4 columns per core; each core computes its
    [S=128, BL=4] slice end-to-end: embedding gather, both RNN scans,
    the vocab projection and log_softmax, writing [512, 50257] fp32.
  - the big [rows, V] matmul folds b_out in as a 33rd contraction row
    (ones row in the feature matrix), runs in bf16, and is done twice:
    pass A feeds exp()+accumulate to get the per-row logsumexp, pass B
    recomputes logits and subtracts L while staging ~2MB output DMAs.
  - no max-subtraction is needed: |logits| <= 33 * 1.0 * (1/sqrt(V)) ~ 0.15,
    so exp() is stable and log_softmax(x) = x - log(sum(exp(x))) exactly.
"""

from contextlib import ExitStack

import ml_dtypes
import numpy as np

import concourse.bass as bass
import concourse.tile as tile
from concourse import bacc
from concourse import mybir
from concourse.bass_utils import run_bass_kernel_spmd
from concourse.masks import make_identity

S, B, V = 128, 32, 50257
EMB, HID = 32, 16
NCORES = 8
BL = B // NCORES          # 4 batch columns per core
R = S * BL                # 512 rows per core (row r = t*BL + b)
KF = 2 * HID + 1          # 33 = contraction rows of the vocab matmul
CHUNK = 512               # vocab columns per matmul (one PSUM bank)
GRP = CHUNK               # vocab columns per ACT/DVE op (one PSUM bank)
NGRP = (V + GRP - 1) // GRP      # 50 (49 full + one 81-col tail)
STAGE = 8 * GRP           # vocab columns per output DMA (4096)
ROWT = R // 128           # 4 row-tiles of 128 rows

_F32 = mybir.dt.float32
_BF16 = mybir.dt.bfloat16
_I32 = mybir.dt.int32
_AF = mybir.ActivationFunctionType
_ALU = mybir.AluOpType

_CACHE: dict = {}


def _emit_rep(nc, tc, pools, aps, rep):
    (const, gather, scr, stats, ostage) = pools
    (embtab, idx, h0lrT, h0rlT, out, wb_sb, wxlr_sb, whlr_sb, blr_sb,
     wxrl_sb, whrl_sb, brl_sb, ident) = aps

    embT = const.tile([EMB, R], _F32, tag="embT")   # emb[t,b]^T at col t*BL+b
    hlr = const.tile([HID, R], _F32, tag="hlr")     # hLR[t]^T at col t*BL+b
    hrl = const.tile([HID, R], _F32, tag="hrl")     # hRL[S-1-t]^T at col t*BL+b
    fb = const.tile([KF, R], _BF16, tag="fb")       # [hLR; hRL_rev; ones]

    with tc.tile_pool(name=f"psum_pro{rep}", bufs=2, space="PSUM") as psum_pro:
        # ---- embedding gather: rows -> [128, EMB] tiles, PE-transpose into embT
        for g in range(R // 128):
            it = gather.tile([128, 1], _I32, tag="it")
            nc.sync.dma_start(it[:], idx[g * 128 : (g + 1) * 128, :])
            en = gather.tile([128, EMB], _F32, tag="en")
            nc.gpsimd.indirect_dma_start(
                out=en[:],
                out_offset=None,
                in_=embtab[:],
                in_offset=bass.IndirectOffsetOnAxis(ap=it[:, :1], axis=0),
            )
            pt = psum_pro.tile([EMB, 128], _F32, tag="pt")
            nc.tensor.transpose(out=pt[:], in_=en[:], identity=ident[:])
            nc.vector.tensor_copy(embT[:, g * 128 : (g + 1) * 128], pt[:])

        # ---- initial hidden states
        nc.sync.dma_start(hlr[:, 0:BL], h0lrT[:])
        nc.sync.dma_start(hrl[:, (S - 1) * BL : S * BL], h0rlT[:])

        # ---- the two scans, interleaved (independent chains)
        # LR step t:  hLR[t] = tanh(Wx@emb[t-1] + Wh@hLR[t-1] + b)
        # RL step k:  hRL[k] = tanh(Wx@emb[S-k] + Wh@hRL[k-1] + b);
        #             hRL[k] lives at col t=S-1-k, hRL[k-1] at col t=S-k.
        for s_ in range(1, S):
            plr = psum_pro.tile([HID, BL], _F32, tag="plr")
            nc.tensor.matmul(
                plr[:], wxlr_sb[:], embT[:, (s_ - 1) * BL : s_ * BL],
                start=True, stop=False,
            )
            nc.tensor.matmul(
                plr[:], whlr_sb[:], hlr[:, (s_ - 1) * BL : s_ * BL],
                start=False, stop=True,
            )
            nc.scalar.activation(
                hlr[:, s_ * BL : (s_ + 1) * BL], plr[:], _AF.Tanh,
                bias=blr_sb[:, 0:1],
            )
            tcol = S - 1 - s_
            prl = psum_pro.tile([HID, BL], _F32, tag="prl")
            nc.tensor.matmul(
                prl[:], wxrl_sb[:], embT[:, (S - s_) * BL : (S - s_ + 1) * BL],
                start=True, stop=False,
            )
            nc.tensor.matmul(
                prl[:], whrl_sb[:], hrl[:, (S - s_) * BL : (S - s_ + 1) * BL],
                start=False, stop=True,
            )
            nc.scalar.activation(
                hrl[:, tcol * BL : (tcol + 1) * BL], prl[:], _AF.Tanh,
                bias=brl_sb[:, 0:1],
            )

        # ---- assemble bf16 feature matrix [33, R] (cast + ones row)
        nc.gpsimd.dma_start(fb[0:HID, :], hlr[:, :])
        nc.gpsimd.dma_start(fb[HID : 2 * HID, :], hrl[:, :])
        nc.vector.memset(fb[2 * HID : KF, :], 1.0)

    with tc.tile_pool(name=f"psum_a{rep}", bufs=3, space="PSUM") as psum_a, \
         tc.tile_pool(name=f"psum_b{rep}", bufs=3, space="PSUM") as psum_b:
        # ---- vocab projection + log_softmax, 4 row-tiles of 128 rows.
        # Software-pipelined: pass A of row-tile i (matmul+exp-accum on ACT)
        # runs concurrently with pass B of row-tile i-1 (matmul+subtract on
        # DVE + staged output DMA). Ops cover GRP=1024 cols (2 PSUM banks)
        # to amortize per-op overhead.
        sums_t = [None] * ROWT
        lse_t = [None] * ROWT

        def mm_group(pool, tag, lhs, c0, n):
            p = pool.tile([128, GRP], _F32, tag=tag, name=tag)
            nc.tensor.matmul(
                p[:, : min(n, CHUNK)], lhs,
                wb_sb[:, c0 : c0 + min(n, CHUNK)], start=True, stop=True,
            )
            if n > CHUNK:
                nc.tensor.matmul(
                    p[:, CHUNK:n], lhs, wb_sb[:, c0 + CHUNK : c0 + n],
                    start=True, stop=True,
                )
            return p

        def emit_a(i, g):
            c0 = g * GRP
            n = min(GRP, V - c0)
            lhs = fb[:, i * 128 : (i + 1) * 128]
            pa = mm_group(psum_a, "pa", lhs, c0, n)
            sc = scr.tile([128, GRP], _BF16, tag="sc")
            nc.scalar.activation(
                sc[:, :n], pa[:, :n], _AF.Exp, accum_out=sums_t[i][:, g : g + 1]
            )

        def emit_lse(i):
            tot = stats.tile([128, 1], _F32, tag="tot")
            nc.vector.tensor_reduce(
                tot[:], sums_t[i][:], axis=mybir.AxisListType.X, op=_ALU.add
            )
            lse_t[i] = stats.tile([128, 1], _F32, tag="lse", name="lse")
            nc.scalar.activation(lse_t[i][:], tot[:], _AF.Ln)

        def emit_b(i, g, ob, off):
            c0 = g * GRP
            n = min(GRP, V - c0)
            lhs = fb[:, i * 128 : (i + 1) * 128]
            pb = mm_group(psum_b, "pb", lhs, c0, n)
            nc.vector.tensor_scalar(
                ob[:, off : off + n], pb[:, :n], lse_t[i][:], None, _ALU.subtract
            )
            return n

        GPS = STAGE // GRP  # groups per output stage
        for i in range(ROWT + 1):
            if i < ROWT:
                sums_t[i] = stats.tile([128, NGRP], _F32, tag="sums", name="sums")
            if i > 0:
                emit_lse(i - 1)
            ob = None
            off = 0
            col = 0
            for g in range(NGRP):
                if i < ROWT:
                    emit_a(i, g)
                if i > 0:
                    if ob is None:
                        ob = ostage.tile([128, STAGE], _F32, tag="ob")
                        off = 0
                        col = g * GRP
                    off += emit_b(i - 1, g, ob, off)
                    if (g + 1) % GPS == 0 or g == NGRP - 1:
                        nc.sync.dma_start(
                            out[(i - 1) * 128 : i * 128, col : col + off],
                            ob[:, :off],
                        )
                        ob = None


def _build_nc(repeats: int = 1) -> bass.Bass:
    nc = bacc.Bacc("TRN2", target_bir_lowering=False, debug=False)

    embtab = nc.dram_tensor("embtab", [V, EMB], _F32, kind="ExternalInput").ap()
    idx = nc.dram_tensor("idx", [R, 1], _I32, kind="ExternalInput").ap()
    wxt_lr = nc.dram_tensor("wxt_lr", [EMB, HID], _F32, kind="ExternalInput").ap()
    wht_lr = nc.dram_tensor("wht_lr", [HID, HID], _F32, kind="ExternalInput").ap()
    b_lr = nc.dram_tensor("b_lr", [HID, 1], _F32, kind="ExternalInput").ap()
    wxt_rl = nc.dram_tensor("wxt_rl", [EMB, HID], _F32, kind="ExternalInput").ap()
    wht_rl = nc.dram_tensor("wht_rl", [HID, HID], _F32, kind="ExternalInput").ap()
    b_rl = nc.dram_tensor("b_rl", [HID, 1], _F32, kind="ExternalInput").ap()
    h0lrT = nc.dram_tensor("h0lrT", [HID, BL], _F32, kind="ExternalInput").ap()
    h0rlT = nc.dram_tensor("h0rlT", [HID, BL], _F32, kind="ExternalInput").ap()
    wb = nc.dram_tensor("wb", [KF, V], _BF16, kind="ExternalInput").ap()
    out = nc.dram_tensor("out", [R, V], _F32, kind="ExternalOutput").ap()

    with tile.TileContext(nc) as tc, ExitStack() as ctx:
        const = ctx.enter_context(tc.tile_pool(name="const", bufs=1))
        gather = ctx.enter_context(tc.tile_pool(name="gather", bufs=2))
        scr = ctx.enter_context(tc.tile_pool(name="scr", bufs=2))
        stats = ctx.enter_context(tc.tile_pool(name="stats", bufs=2))
        ostage = ctx.enter_context(tc.tile_pool(name="ostage", bufs=3))

        # ---- constants into SBUF (loaded once)
        wb_sb = const.tile([KF, V], _BF16)
        nc.sync.dma_start(wb_sb[:], wb[:])
        wxlr_sb = const.tile([EMB, HID], _F32)
        nc.sync.dma_start(wxlr_sb[:], wxt_lr[:])
        whlr_sb = const.tile([HID, HID], _F32)
        nc.sync.dma_start(whlr_sb[:], wht_lr[:])
        blr_sb = const.tile([HID, 1], _F32)
        nc.sync.dma_start(blr_sb[:], b_lr[:])
        wxrl_sb = const.tile([EMB, HID], _F32)
        nc.sync.dma_start(wxrl_sb[:], wxt_rl[:])
        whrl_sb = const.tile([HID, HID], _F32)
        nc.sync.dma_start(whrl_sb[:], wht_rl[:])
        brl_sb = const.tile([HID, 1], _F32)
        nc.sync.dma_start(brl_sb[:], b_rl[:])
        ident = const.tile([128, 128], _F32)
        make_identity(nc, ident[:])

        pools = (const, gather, scr, stats, ostage)
        aps = (embtab, idx, h0lrT, h0rlT, out, wb_sb, wxlr_sb, whlr_sb,
               blr_sb, wxrl_sb, whrl_sb, brl_sb, ident)
        for rep in range(repeats):
            _emit_rep(nc, tc, pools, aps, rep)

    nc.compile()
    return nc


def _get_nc(repeats: int = 1) -> bass.Bass:
    key = f"nc{repeats}"
    if key not in _CACHE:
        _CACHE[key] = _build_nc(repeats)
    return _CACHE[key]


def _make_in_maps(inputs: dict) -> list[dict]:
    ib = np.asarray(inputs["input_batch"]).astype(np.int32)          # [S, B]
    emb = np.ascontiguousarray(np.asarray(inputs["embedding"], dtype=np.float32))
    w_lr = np.asarray(inputs["W_lr"], dtype=np.float32)              # [HID, EMB+HID]
    w_rl = np.asarray(inputs["W_rl"], dtype=np.float32)
    b_lr = np.asarray(inputs["b_lr"], dtype=np.float32)
    b_rl = np.asarray(inputs["b_rl"], dtype=np.float32)
    w_out = np.asarray(inputs["W_out"], dtype=np.float32)            # [V, 2*HID]
    b_out = np.asarray(inputs["b_out"], dtype=np.float32)
    h0_lr = np.asarray(inputs["h0_lr"], dtype=np.float32)            # [B, HID]
    h0_rl = np.asarray(inputs["h0_rl"], dtype=np.float32)

    shared = {
        "embtab": emb,
        "wxt_lr": np.ascontiguousarray(w_lr[:, :EMB].T),
        "wht_lr": np.ascontiguousarray(w_lr[:, EMB:].T),
        "b_lr": np.ascontiguousarray(b_lr[:, None]),
        "wxt_rl": np.ascontiguousarray(w_rl[:, :EMB].T),
        "wht_rl": np.ascontiguousarray(w_rl[:, EMB:].T),
        "b_rl": np.ascontiguousarray(b_rl[:, None]),
        "wb": np.ascontiguousarray(
            np.concatenate([w_out.T, b_out[None, :]], axis=0)
        ).astype(ml_dtypes.bfloat16),
    }
    in_maps = []
    for c in range(NCORES):
        cols = slice(c * BL, (c + 1) * BL)
        in_maps.append(
            dict(
                shared,
                idx=np.ascontiguousarray(ib[:, cols].reshape(R, 1)),
                h0lrT=np.ascontiguousarray(h0_lr[cols, :].T),
                h0rlT=np.ascontiguousarray(h0_rl[cols, :].T),
            )
        )
    return in_maps


def _run(inputs: dict, repeats: int = 1, **spmd_kwargs):
    nc = _get_nc(repeats)
    res = run_bass_kernel_spmd(
        nc, _make_in_maps(inputs), core_ids=list(range(NCORES)), **spmd_kwargs
    )
    outs = [res.results[c]["out"].reshape(S, BL, V) for c in range(NCORES)]
    return np.concatenate(outs, axis=1), res


def kernel(**inputs) -> np.ndarray:
    full, _ = _run(inputs)
    return full


# revision 11
# speedup vs baseline: 1.1146x; 1.0460x over previous
"""BiRNN LM kernel for Trainium2, 8 NeuronCores.

Strategy (data-parallel over batch):
  - batch B=32 is split 4 columns per core; each core computes its
    [S=128, BL=4] slice end-to-end: embedding gather, both RNN scans,
    the vocab projection and log_softmax, writing [512, 50257] fp32.
  - the big [rows, V] matmul folds b_out in as a 33rd contraction row
    (ones row in the feature matrix), runs in bf16, and is done twice:
    pass A feeds exp()+accumulate to get the per-row logsumexp, pass B
    recomputes logits and subtracts L while staging ~2MB output DMAs.
  - no max-subtraction is needed: |logits| <= 33 * 1.0 * (1/sqrt(V)) ~ 0.15,
    so exp() is stable and log_softmax(x) = x - log(sum(exp(x))) exactly.
"""

from contextlib import ExitStack

import ml_dtypes
import numpy as np

import concourse.bass as bass
import concourse.tile as tile
from concourse import bacc
from concourse import mybir
from concourse.bass_utils import run_bass_kernel_spmd
from concourse.masks import make_identity

S, B, V = 128, 32, 50257
EMB, HID = 32, 16
NCORES = 8
BL = B // NCORES          # 4 batch columns per core
R = S * BL                # 512 rows per core (row r = t*BL + b)
KF = 2 * HID + 1          # 33 = contraction rows of the vocab matmul
CHUNK = 512               # vocab columns per matmul (one PSUM bank)
GRP = 2 * CHUNK           # vocab columns per ACT/DVE op (2 PSUM banks)
NGRP = (V + GRP - 1) // GRP      # 50 (49 full + one 81-col tail)
STAGE = 4 * GRP           # vocab columns per output DMA (4096)
ROWT = R // 128           # 4 row-tiles of 128 rows

_F32 = mybir.dt.float32
_BF16 = mybir.dt.bfloat16
_I32 = mybir.dt.int32
_AF = mybir.ActivationFunctionType
_ALU = mybir.AluOpType

_CACHE: dict = {}


def _emit_rep(nc, tc, pools, aps, rep):
    (const, gather, scr, stats, ostage) = pools
    (embtab, idx, h0lrT, h0rlT, out, wb, wb_sb, wxlr_sb, whlr_sb, blr_sb,
     wxrl_sb, whrl_sb, brl_sb, ident) = aps

    embT = const.tile([EMB, R], _F32, tag="embT")   # emb[t,b]^T at col t*BL+b
    hlr = const.tile([HID, R], _F32, tag="hlr")     # hLR[t]^T at col t*BL+b
    hrl = const.tile([HID, R], _F32, tag="hrl")     # hRL[S-1-t]^T at col t*BL+b
    fb = const.tile([KF, R], _BF16, tag="fb")       # [hLR; hRL_rev; ones]

    with tc.tile_pool(name=f"psum_pro{rep}", bufs=2, space="PSUM") as psum_pro:
        # ---- embedding gather: rows -> [128, EMB] tiles, PE-transpose into embT
        for g in range(R // 128):
            it = gather.tile([128, 1], _I32, tag="it")
            nc.sync.dma_start(it[:], idx[g * 128 : (g + 1) * 128, :])
            en = gather.tile([128, EMB], _F32, tag="en")
            nc.gpsimd.indirect_dma_start(
                out=en[:],
                out_offset=None,
                in_=embtab[:],
                in_offset=bass.IndirectOffsetOnAxis(ap=it[:, :1], axis=0),
            )
            pt = psum_pro.tile([EMB, 128], _F32, tag="pt")
            nc.tensor.transpose(out=pt[:], in_=en[:], identity=ident[:])
            nc.vector.tensor_copy(embT[:, g * 128 : (g + 1) * 128], pt[:])

        if rep == 0:
            nc.gpsimd.dma_start(wb_sb[:], wb[:])

        # ---- initial hidden states
        nc.sync.dma_start(hlr[:, 0:BL], h0lrT[:])
        nc.sync.dma_start(hrl[:, (S - 1) * BL : S * BL], h0rlT[:])

        # ---- the two scans, interleaved (independent chains)
        # LR step t:  hLR[t] = tanh(Wx@emb[t-1] + Wh@hLR[t-1] + b)
        # RL step k:  hRL[k] = tanh(Wx@emb[S-k] + Wh@hRL[k-1] + b);
        #             hRL[k] lives at col t=S-1-k, hRL[k-1] at col t=S-k.
        for s_ in range(1, S):
            plr = psum_pro.tile([HID, BL], _F32, tag="plr")
            nc.tensor.matmul(
                plr[:], wxlr_sb[:], embT[:, (s_ - 1) * BL : s_ * BL],
                start=True, stop=False,
            )
            nc.tensor.matmul(
                plr[:], whlr_sb[:], hlr[:, (s_ - 1) * BL : s_ * BL],
                start=False, stop=True,
            )
            nc.scalar.activation(
                hlr[:, s_ * BL : (s_ + 1) * BL], plr[:], _AF.Tanh,
                bias=blr_sb[:, 0:1],
            )
            tcol = S - 1 - s_
            prl = psum_pro.tile([HID, BL], _F32, tag="prl")
            nc.tensor.matmul(
                prl[:], wxrl_sb[:], embT[:, (S - s_) * BL : (S - s_ + 1) * BL],
                start=True, stop=False,
            )
            nc.tensor.matmul(
                prl[:], whrl_sb[:], hrl[:, (S - s_) * BL : (S - s_ + 1) * BL],
                start=False, stop=True,
            )
            nc.scalar.activation(
                hrl[:, tcol * BL : (tcol + 1) * BL], prl[:], _AF.Tanh,
                bias=brl_sb[:, 0:1],
            )

        # ---- assemble bf16 feature matrix [33, R] (cast + ones row)
        nc.gpsimd.dma_start(fb[0:HID, :], hlr[:, :])
        nc.gpsimd.dma_start(fb[HID : 2 * HID, :], hrl[:, :])
        nc.vector.memset(fb[2 * HID : KF, :], 1.0)

    with tc.tile_pool(name=f"psum_a{rep}", bufs=2, space="PSUM") as psum_a, \
         tc.tile_pool(name=f"psum_b{rep}", bufs=2, space="PSUM") as psum_b:
        # ---- vocab projection + log_softmax, 4 row-tiles of 128 rows.
        # Software-pipelined: pass A of row-tile i (matmul+exp-accum on ACT)
        # runs concurrently with pass B of row-tile i-1 (matmul+subtract on
        # DVE + staged output DMA). Ops cover GRP=1024 cols (2 PSUM banks)
        # to amortize per-op overhead.
        sums_t = [None] * ROWT
        lse_t = [None] * ROWT

        def mm_chunk(p, lhsL, lhsH, pcol, c0, n):
            # two concurrent M=64 matmuls on disjoint PE column strips
            nc.tensor.matmul(
                p[0:64, pcol : pcol + n], lhsL, wb_sb[:, c0 : c0 + n],
                start=True, stop=True, tile_position=(0, 0),
            )
            nc.tensor.matmul(
                p[64:128, pcol : pcol + n], lhsH, wb_sb[:, c0 : c0 + n],
                start=True, stop=True, tile_position=(0, 64),
            )

        def mm_group(pool, tag, i, c0, n):
            lhsL = fb[:, i * 128 : i * 128 + 64]
            lhsH = fb[:, i * 128 + 64 : (i + 1) * 128]
            p = pool.tile([128, GRP], _F32, tag=tag, name=tag)
            mm_chunk(p, lhsL, lhsH, 0, c0, min(n, CHUNK))
            if n > CHUNK:
                mm_chunk(p, lhsL, lhsH, CHUNK, c0 + CHUNK, n - CHUNK)
            return p

        def emit_a(i, g):
            c0 = g * GRP
            n = min(GRP, V - c0)
            pa = mm_group(psum_a, "pa", i, c0, n)
            sc = scr.tile([128, GRP], _BF16, tag="sc")
            nc.scalar.activation(
                sc[:, :n], pa[:, :n], _AF.Exp, accum_out=sums_t[i][:, g : g + 1]
            )

        def emit_lse(i):
            tot = stats.tile([128, 1], _F32, tag="tot")
            nc.vector.tensor_reduce(
                tot[:], sums_t[i][:], axis=mybir.AxisListType.X, op=_ALU.add
            )
            lse_t[i] = stats.tile([128, 1], _F32, tag="lse", name="lse")
            nc.scalar.activation(lse_t[i][:], tot[:], _AF.Ln)

        def emit_b(i, g, ob, off):
            c0 = g * GRP
            n = min(GRP, V - c0)
            pb = mm_group(psum_b, "pb", i, c0, n)
            nc.vector.tensor_scalar(
                ob[:, off : off + n], pb[:, :n], lse_t[i][:], None, _ALU.subtract
            )
            return n

        GPS = STAGE // GRP  # groups per output stage
        for i in range(ROWT + 1):
            if i < ROWT:
                sums_t[i] = stats.tile([128, NGRP], _F32, tag="sums", name="sums")
            if i > 0:
                emit_lse(i - 1)
            ob = None
            off = 0
            col = 0
            for g in range(NGRP):
                if i < ROWT:
                    emit_a(i, g)
                if i > 0:
                    if ob is None:
                        ob = ostage.tile([128, STAGE], _F32, tag="ob")
                        off = 0
                        col = g * GRP
                    off += emit_b(i - 1, g, ob, off)
                    if (g + 1) % GPS == 0 or g == NGRP - 1:
                        nc.sync.dma_start(
                            out[(i - 1) * 128 : i * 128, col : col + off],
                            ob[:, :off],
                        )
                        ob = None


def _build_nc(repeats: int = 1) -> bass.Bass:
    nc = bacc.Bacc("TRN2", target_bir_lowering=False, debug=False)

    embtab = nc.dram_tensor("embtab", [V, EMB], _F32, kind="ExternalInput").ap()
    idx = nc.dram_tensor("idx", [R, 1], _I32, kind="ExternalInput").ap()
    wxt_lr = nc.dram_tensor("wxt_lr", [EMB, HID], _F32, kind="ExternalInput").ap()
    wht_lr = nc.dram_tensor("wht_lr", [HID, HID], _F32, kind="ExternalInput").ap()
    b_lr = nc.dram_tensor("b_lr", [HID, 1], _F32, kind="ExternalInput").ap()
    wxt_rl = nc.dram_tensor("wxt_rl", [EMB, HID], _F32, kind="ExternalInput").ap()
    wht_rl = nc.dram_tensor("wht_rl", [HID, HID], _F32, kind="ExternalInput").ap()
    b_rl = nc.dram_tensor("b_rl", [HID, 1], _F32, kind="ExternalInput").ap()
    h0lrT = nc.dram_tensor("h0lrT", [HID, BL], _F32, kind="ExternalInput").ap()
    h0rlT = nc.dram_tensor("h0rlT", [HID, BL], _F32, kind="ExternalInput").ap()
    wb = nc.dram_tensor("wb", [KF, V], _BF16, kind="ExternalInput").ap()
    out = nc.dram_tensor("out", [R, V], _F32, kind="ExternalOutput").ap()

    with tile.TileContext(nc) as tc, ExitStack() as ctx:
        const = ctx.enter_context(tc.tile_pool(name="const", bufs=1))
        gather = ctx.enter_context(tc.tile_pool(name="gather", bufs=2))
        scr = ctx.enter_context(tc.tile_pool(name="scr", bufs=2))
        stats = ctx.enter_context(tc.tile_pool(name="stats", bufs=2))
        ostage = ctx.enter_context(tc.tile_pool(name="ostage", bufs=3))

        # ---- constants into SBUF (loaded once)
        wb_sb = const.tile([KF, V], _BF16)
        wxlr_sb = const.tile([EMB, HID], _F32)
        nc.sync.dma_start(wxlr_sb[:], wxt_lr[:])
        whlr_sb = const.tile([HID, HID], _F32)
        nc.sync.dma_start(whlr_sb[:], wht_lr[:])
        blr_sb = const.tile([HID, 1], _F32)
        nc.sync.dma_start(blr_sb[:], b_lr[:])
        wxrl_sb = const.tile([EMB, HID], _F32)
        nc.sync.dma_start(wxrl_sb[:], wxt_rl[:])
        whrl_sb = const.tile([HID, HID], _F32)
        nc.sync.dma_start(whrl_sb[:], wht_rl[:])
        brl_sb = const.tile([HID, 1], _F32)
        nc.sync.dma_start(brl_sb[:], b_rl[:])
        ident = const.tile([128, 128], _F32)
        make_identity(nc, ident[:])

        pools = (const, gather, scr, stats, ostage)
        aps = (embtab, idx, h0lrT, h0rlT, out, wb, wb_sb, wxlr_sb, whlr_sb,
               blr_sb, wxrl_sb, whrl_sb, brl_sb, ident)
        for rep in range(repeats):
            _emit_rep(nc, tc, pools, aps, rep)

    nc.compile()
    return nc


def _get_nc(repeats: int = 1) -> bass.Bass:
    key = f"nc{repeats}"
    if key not in _CACHE:
        _CACHE[key] = _build_nc(repeats)
    return _CACHE[key]


def _make_in_maps(inputs: dict) -> list[dict]:
    ib = np.asarray(inputs["input_batch"]).astype(np.int32)          # [S, B]
    emb = np.ascontiguousarray(np.asarray(inputs["embedding"], dtype=np.float32))
    w_lr = np.asarray(inputs["W_lr"], dtype=np.float32)              # [HID, EMB+HID]
    w_rl = np.asarray(inputs["W_rl"], dtype=np.float32)
    b_lr = np.asarray(inputs["b_lr"], dtype=np.float32)
    b_rl = np.asarray(inputs["b_rl"], dtype=np.float32)
    w_out = np.asarray(inputs["W_out"], dtype=np.float32)            # [V, 2*HID]
    b_out = np.asarray(inputs["b_out"], dtype=np.float32)
    h0_lr = np.asarray(inputs["h0_lr"], dtype=np.float32)            # [B, HID]
    h0_rl = np.asarray(inputs["h0_rl"], dtype=np.float32)

    shared = {
        "embtab": emb,
        "wxt_lr": np.ascontiguousarray(w_lr[:, :EMB].T),
        "wht_lr": np.ascontiguousarray(w_lr[:, EMB:].T),
        "b_lr": np.ascontiguousarray(b_lr[:, None]),
        "wxt_rl": np.ascontiguousarray(w_rl[:, :EMB].T),
        "wht_rl": np.ascontiguousarray(w_rl[:, EMB:].T),
        "b_rl": np.ascontiguousarray(b_rl[:, None]),
        "wb": np.ascontiguousarray(
            np.concatenate([w_out.T, b_out[None, :]], axis=0)
        ).astype(ml_dtypes.bfloat16),
    }
    in_maps = []
    for c in range(NCORES):
        cols = slice(c * BL, (c + 1) * BL)
        in_maps.append(
            dict(
                shared,
                idx=np.ascontiguousarray(ib[:, cols].reshape(R, 1)),
                h0lrT=np.ascontiguousarray(h0_lr[cols, :].T),
                h0rlT=np.ascontiguousarray(h0_rl[cols, :].T),
            )
        )
    return in_maps


def _run(inputs: dict, repeats: int = 1, **spmd_kwargs):
    nc = _get_nc(repeats)
    res = run_bass_kernel_spmd(
        nc, _make_in_maps(inputs), core_ids=list(range(NCORES)), **spmd_kwargs
    )
    outs = [res.results[c]["out"].reshape(S, BL, V) for c in range(NCORES)]
    return np.concatenate(outs, axis=1), res


def kernel(**inputs) -> np.ndarray:
    full, _ = _run(inputs)
    return full
